# revision 16
# baseline (speedup 1.0000x reference)
"""Multi-head causal attention (B=4, S=2048, D=1024, H=16) on 8 TRN2 cores.

Sharding: core c handles batch c//2 and head-group c%2 (8 heads = 512 dims).
Each core computes its group's QKV projections, causal attention, and two
partial O-projections (out0 = d-blocks 0..2, out1 = d-block 3); the host
sums the four partials per batch.

v2 redesign (baseline was 653us):
- bf16 inputs + weights (halves input DMA); q/k activations kept fp32r.
- attention loops q-chunk OUTER, kv-pair inner. PSUM: 2 score bufs
  [128,1024] + 2 oT accumulators [128,512] + 2 po bufs = 8 banks.
- V tiles carry a shared 64-wide ones block per head pair
  ([even | ones | odd] x 4); the AV matmul then yields 64 numerator rows
  and 64 replicated denominator rows in one pass, so the epilogue is just
  reciprocal_approx_fast + one tensor multiply. No gpsimd broadcast, no
  slow DVE reciprocal, no partition-shift DMA.
- exp always full [128,1024] (stale/garbage columns are never read by AV).
- V-projection runs as PE filler inside head 0; O-projection is split
  out0/out1 and interleaved into heads 6-7 so the PE stays dense enough
  to hold its 2.4GHz p-state (it idles down to 1.2GHz otherwise).
"""

import numpy as np

import concourse.bass as bass
import concourse.mybir as mybir
import concourse.tile as tile
from concourse import bacc
from concourse.bass_utils import run_bass_kernel_spmd

F32 = mybir.dt.float32
F32R = mybir.dt.float32r
BF16 = mybir.dt.bfloat16
EXP = mybir.ActivationFunctionType.Exp

B, S, D = 4, 2048, 1024
G = 512          # dims per head group
NT = S // 128    # 16 token tiles
NEG = -1.0e30


def build():
    nc = bacc.Bacc("TRN2", num_devices=8)

    xq = nc.dram_tensor("xq", [D, S], BF16, kind="ExternalInput")
    xk = nc.dram_tensor("xk", [D, S], BF16, kind="ExternalInput")
    xv = nc.dram_tensor("xv", [D, S], BF16, kind="ExternalInput")
    wq = nc.dram_tensor("wq", [D, G], BF16, kind="ExternalInput")
    wk = nc.dram_tensor("wk", [D, G], BF16, kind="ExternalInput")
    wv = nc.dram_tensor("wv", [D, G], BF16, kind="ExternalInput")
    wo = nc.dram_tensor("wo", [G, D], BF16, kind="ExternalInput")
    mb_d = nc.dram_tensor("mb", [128, 128], F32, kind="ExternalInput")
    out0_d = nc.dram_tensor("out0", [S, D], F32, kind="ExternalOutput")
    out1_d = nc.dram_tensor("out1", [S, D], F32, kind="ExternalOutput")

    with tile.TileContext(nc) as tc:
        with tc.tile_pool(name="persist", bufs=1) as persist:
            qT = persist.tile([128, 4, S], BF16, tag="qT", name="qT")
            kT = persist.tile([128, 4, S], BF16, tag="kT", name="kT")
            # per token tile: 4 groups of [even(64) | ones(64) | odd(64)]
            vA = persist.tile([128, NT, 768], BF16, tag="vA", name="vA")
            xT = persist.tile([128, 4, S], BF16, tag="xT", name="xT")
            xv_sb = persist.tile([128, 8, S], BF16, tag="xv", name="xv_sb")
            wv_sb = persist.tile([128, 8, G], BF16, tag="wv", name="wv_sb")
            wo_sb = persist.tile([128, 4, D], BF16, tag="wo", name="wo_sb")
            mb = persist.tile([128, 128], F32, tag="mb", name="mb")

            nc.gpsimd.memset(
                vA.rearrange("p t (q c) -> p (t q) c", c=192)[:, :, 64:128], 1.0
            )

            # ---------------- phase 1: Q/K projections ----------------
            with (
                tc.tile_pool(name="p1x", bufs=3) as p1x,
                tc.tile_pool(name="p1w", bufs=2) as p1w,
                tc.tile_pool(name="ps1", bufs=4, space="PSUM") as ps1,
            ):
                with nc.named_scope("proj"):
                    for kind, xd, wd, dest in (("q", xq, wq, qT), ("k", xk, wk, kT)):
                        w_sb = p1w.tile([128, 8, G], BF16, tag="w", name=f"w_{kind}")
                        nc.sync.dma_start(
                            out=w_sb, in_=wd.ap().rearrange("(a p) n -> p a n", p=128)
                        )
                        for tci in range(4):
                            xt = p1x.tile([128, 8, 512], BF16, tag="xt",
                                          name=f"xt_{kind}{tci}")
                            nc.sync.dma_start(
                                out=xt,
                                in_=xd.ap()[:, 512 * tci:512 * tci + 512]
                                .rearrange("(a p) t -> p a t", p=128),
                            )
                            for dq in range(4):
                                acc = ps1.tile([128, 512], F32, tag="pj",
                                               name=f"pj_{kind}{tci}{dq}")
                                for dm in range(8):
                                    nc.tensor.matmul(
                                        acc,
                                        w_sb[:, dm, 128 * dq:128 * dq + 128],
                                        xt[:, dm, :],
                                        start=(dm == 0), stop=(dm == 7),
                                    )
                                nc.scalar.copy(
                                    dest[:, dq, 512 * tci:512 * tci + 512], acc
                                )
                    # V inputs for the in-attention V-projection filler
                    nc.sync.dma_start(
                        out=wv_sb, in_=wv.ap().rearrange("(a p) n -> p a n", p=128)
                    )
                    nc.sync.dma_start(
                        out=xv_sb, in_=xv.ap().rearrange("(a p) t -> p a t", p=128)
                    )
                    nc.sync.dma_start(
                        out=wo_sb, in_=wo.ap().rearrange("(a p) n -> p a n", p=128)
                    )
                    nc.sync.dma_start(out=mb, in_=mb_d.ap())

            # ---------------- phase 2: attention (+ fillers) ----------------
            with (
                tc.tile_pool(name="p2", bufs=1) as p2,
                tc.tile_pool(name="ps2", bufs=1, space="PSUM") as ps2,
            ):
                with nc.named_scope("attn"):
                    prev_mm = [None]

                    def chain(bi):
                        if prev_mm[0] is not None:
                            tile.add_dep_helper(
                                bi.ins, prev_mm[0].ins, sync=False,
                                reason="attn PE order",
                            )
                        prev_mm[0] = bi

                    def vproj(t):
                        # V-proj of token tile t -> vA[:, t, dims blocks]
                        acc = ps2.tile([128, 512], F32, tag="po", bufs=2,
                                       name=f"pv{t}")
                        for dm in range(8):
                            chain(nc.tensor.matmul(
                                acc,
                                xv_sb[:, dm, 128 * t:128 * t + 128],
                                wv_sb[:, dm, :],
                                start=(dm == 0), stop=(dm == 7),
                            ))
                        accv = acc.rearrange("p (q two c) -> p q two c", two=2, c=64)
                        vAt = vA[:, t, :].rearrange("p (q g c) -> p q g c", g=3, c=64)
                        nc.scalar.copy(vAt[:, :, 0, :], accv[:, :, 0, :])
                        nc.scalar.copy(vAt[:, :, 2, :], accv[:, :, 1, :])

                    # O-projection group queues: ("0", ii, n) uses d=0..2 into
                    # out0; ("1", ii, n) uses d=3 into out1.
                    po0_q = [("0", ii, n) for ii in range(NT) for n in range(2)]
                    po1_q = []  # filled as h7 epilogues complete

                    def po_group(which, ii, n):
                        acc = ps2.tile([128, 512], F32, tag="po", bufs=2,
                                       name=f"po{which}_{ii}_{n}")
                        ds = (0, 1, 2) if which == "0" else (3,)
                        for z, d_ in enumerate(ds):
                            chain(nc.tensor.matmul(
                                acc,
                                xT[:, d_, 128 * ii:128 * ii + 128],
                                wo_sb[:, d_, 512 * n:512 * n + 512],
                                start=(z == 0), stop=(z == len(ds) - 1),
                            ))
                        ob = p2.tile([128, 512], F32, tag="ob", bufs=3,
                                     name=f"ob{which}_{ii}_{n}")
                        nc.vector.tensor_scalar_add(ob, acc, 0.0)
                        dst = out0_d if which == "0" else out1_d
                        nc.sync.dma_start(
                            out=dst.ap()[128 * ii:128 * ii + 128,
                                         512 * n:512 * n + 512],
                            in_=ob,
                        )

                    def po_fill(budget):
                        while budget > 0 and (po0_q or po1_q):
                            which, ii, n = (po0_q or po1_q).pop(0)
                            po_group(which, ii, n)
                            budget -= 1

                    for h in range(8):
                        d, par = h // 2, h % 2
                        off = 64 * par
                        kTh = kT[off:off + 64, d, :]
                        qTh = qT[off:off + 64, d, :]
                        for j in (3, 2, 1, 0):
                            oTj = ps2.tile([128, 512], F32, tag=f"oT{j % 2}",
                                           name=f"oT_{h}_{j}")
                            U = 2 * j + 2
                            pts = {}

                            def av(u):
                                for half in range(2):
                                    kv = 2 * u + half
                                    q0 = max(0, 128 * kv - 512 * j)
                                    lhsT = vA[:, kv,
                                              192 * d + off:192 * d + off + 128]
                                    chain(nc.tensor.matmul(
                                        oTj[:, q0:512],
                                        lhsT,
                                        pts[u][:, 512 * half + q0:512 * half + 512],
                                        start=(kv == 0), stop=(kv == 4 * j + 3),
                                    ))

                            for u in range(U):
                                sb = ps2.tile([128, 1024], F32, tag=f"S{u % 2}",
                                              name=f"s_{h}_{j}_{u}")
                                for half in range(2):
                                    kv = 2 * u + half
                                    q0 = max(0, 128 * kv - 512 * j)
                                    chain(nc.tensor.matmul(
                                        sb[:, 512 * half + q0:512 * half + 512],
                                        kTh[:, 128 * kv:128 * kv + 128],
                                        qTh[:, 512 * j + q0:512 * j + 512],
                                        start=True, stop=True,
                                    ))
                                if u >= U - 2:  # diagonal unit: mask adds
                                    for half in range(2):
                                        kv = 2 * u + half
                                        q0 = 128 * kv - 512 * j
                                        nc.vector.tensor_add(
                                            sb[:, 512 * half + q0:
                                               512 * half + q0 + 128],
                                            sb[:, 512 * half + q0:
                                               512 * half + q0 + 128],
                                            mb,
                                        )
                                if h == 0 and j == 3:
                                    # V-proj filler: 2 tiles per unit of the
                                    # first (8-unit) chunk, 2 units ahead of
                                    # the lag-2 AV consumer
                                    vproj(2 * u)
                                    vproj(2 * u + 1)
                                pt = p2.tile([128, 1024], BF16, tag="pt", bufs=6,
                                             name=f"pt_{h}_{j}_{u}")
                                nc.scalar.activation(pt, sb, EXP, scale=0.125)
                                pts[u] = pt
                                if u >= 2:
                                    av(u - 2)
                                if h >= 6:
                                    po_fill(2)
                            av(U - 2)
                            av(U - 1)

                            # epilogue: normalize chunk j of head h into xT.
                            # recip_approx_fast requires SBUF src at partition
                            # base 0, so stage the denominator rows first.
                            den = p2.tile([64, 512], F32, tag="den", bufs=2,
                                          name=f"den_{h}_{j}")
                            rbc = p2.tile([64, 512], F32, tag="rbc", bufs=2,
                                          name=f"rbc_{h}_{j}")
                            dlo, dhi = ((64, 128) if par == 0 else (0, 64))
                            nc.vector.tensor_scalar_add(
                                den, oTj[dlo:dhi, :], 0.0)
                            nc.vector.reciprocal_approx_fast(rbc, den)
                            xlo, xhi = ((0, 64) if par == 0 else (64, 128))
                            nc.vector.tensor_mul(
                                xT[xlo:xhi, d, 512 * j:512 * j + 512],
                                oTj[xlo:xhi, :], rbc,
                            )
                            if h == 7:
                                # out1 groups for this chunk now computable
                                po1_q.extend(
                                    ("1", ii, n)
                                    for ii in range(4 * j, 4 * j + 4)
                                    for n in range(2)
                                )

                    # ---------------- phase 3: O-projection tail ----------------
                    with nc.named_scope("oproj"):
                        while po0_q or po1_q:
                            which, ii, n = (po0_q or po1_q).pop(0)
                            po_group(which, ii, n)

    nc.compile()
    return nc


_NC = None


def _get_nc():
    global _NC
    if _NC is None:
        _NC = build()
    return _NC


def _make_in_maps(q, k, v, w_q, w_k, w_v, w_o):
    bf = mybir.dt.np(BF16)
    col = np.arange(128)[None, :]
    row = np.arange(128)[:, None]
    mbig = np.where(col >= row, 0.0, NEG).astype(np.float32)

    xqT = [np.ascontiguousarray(np.asarray(q[b]).T).astype(bf) for b in range(B)]
    xkT = [np.ascontiguousarray(np.asarray(k[b]).T).astype(bf) for b in range(B)]
    xvT = [np.ascontiguousarray(np.asarray(v[b]).T).astype(bf) for b in range(B)]
    wqT = [np.ascontiguousarray(np.asarray(w_q[G * g:G * g + G, :]).T).astype(bf)
           for g in range(2)]
    wkT = [np.ascontiguousarray(np.asarray(w_k[G * g:G * g + G, :]).T).astype(bf)
           for g in range(2)]
    wvT = [np.ascontiguousarray(np.asarray(w_v[G * g:G * g + G, :]).T).astype(bf)
           for g in range(2)]
    woT = [np.ascontiguousarray(np.asarray(w_o[:, G * g:G * g + G]).T).astype(bf)
           for g in range(2)]

    in_maps = []
    for c in range(8):
        b, g = c // 2, c % 2
        in_maps.append({
            "xq": xqT[b], "xk": xkT[b], "xv": xvT[b],
            "wq": wqT[g], "wk": wkT[g], "wv": wvT[g], "wo": woT[g],
            "mb": mbig,
        })
    return in_maps


def _gather(results):
    out = np.empty((B, S, D), np.float32)
    for b in range(B):
        out[b] = (results[2 * b]["out0"] + results[2 * b]["out1"]
                  + results[2 * b + 1]["out0"] + results[2 * b + 1]["out1"])
    return out


def run_kernel(inputs, trace=False, tmpdir=None):
    """Run on 8 cores; returns (out, BassKernelResults)."""
    in_maps = _make_in_maps(
        inputs["q"], inputs["k"], inputs["v"],
        inputs["w_q"], inputs["w_k"], inputs["w_v"], inputs["w_o"],
    )
    res = run_bass_kernel_spmd(
        _get_nc(), in_maps, core_ids=list(range(8)), trace=trace, tmpdir=tmpdir
    )
    return _gather(res.results), res


def kernel(**inputs) -> np.ndarray:
    out, _ = run_kernel(inputs)
    return out


# revision 17
# speedup vs baseline: 1.0322x; 1.0322x over previous
"""Multi-head causal attention (B=4, S=2048, D=1024, H=16) on 8 TRN2 cores.

Sharding: core c handles batch c//2 and head-group c%2 (8 heads = 512 dims).
Each core computes its group's QKV projections, causal attention, and two
partial O-projections (out0 = d-blocks 0..2, out1 = d-block 3); the host
sums the four partials per batch.

v2 redesign (baseline was 653us):
- bf16 inputs + weights (halves input DMA); q/k activations kept fp32r.
- attention loops q-chunk OUTER, kv-pair inner. PSUM: 2 score bufs
  [128,1024] + 2 oT accumulators [128,512] + 2 po bufs = 8 banks.
- V tiles carry a shared 64-wide ones block per head pair
  ([even | ones | odd] x 4); the AV matmul then yields 64 numerator rows
  and 64 replicated denominator rows in one pass, so the epilogue is just
  reciprocal_approx_fast + one tensor multiply. No gpsimd broadcast, no
  slow DVE reciprocal, no partition-shift DMA.
- exp always full [128,1024] (stale/garbage columns are never read by AV).
- V-projection runs as PE filler inside head 0; O-projection is split
  out0/out1 and interleaved into heads 6-7 so the PE stays dense enough
  to hold its 2.4GHz p-state (it idles down to 1.2GHz otherwise).
"""

import numpy as np

import concourse.bass as bass
import concourse.mybir as mybir
import concourse.tile as tile
from concourse import bacc
from concourse.bass_utils import run_bass_kernel_spmd

F32 = mybir.dt.float32
F32R = mybir.dt.float32r
BF16 = mybir.dt.bfloat16
EXP = mybir.ActivationFunctionType.Exp

B, S, D = 4, 2048, 1024
G = 512          # dims per head group
NT = S // 128    # 16 token tiles
NEG = -1.0e30


def build():
    nc = bacc.Bacc("TRN2", num_devices=8)

    xq = nc.dram_tensor("xq", [D, S], BF16, kind="ExternalInput")
    xk = nc.dram_tensor("xk", [D, S], BF16, kind="ExternalInput")
    xv = nc.dram_tensor("xv", [D, S], BF16, kind="ExternalInput")
    wq = nc.dram_tensor("wq", [D, G], BF16, kind="ExternalInput")
    wk = nc.dram_tensor("wk", [D, G], BF16, kind="ExternalInput")
    wv = nc.dram_tensor("wv", [D, G], BF16, kind="ExternalInput")
    wo = nc.dram_tensor("wo", [G, D], BF16, kind="ExternalInput")
    mb_d = nc.dram_tensor("mb", [128, 128], F32, kind="ExternalInput")
    out0_d = nc.dram_tensor("out0", [S, D], F32, kind="ExternalOutput")
    out1_d = nc.dram_tensor("out1", [S, D], F32, kind="ExternalOutput")

    with tile.TileContext(nc) as tc:
        with tc.tile_pool(name="persist", bufs=1) as persist:
            qT = persist.tile([128, 4, S], BF16, tag="qT", name="qT")
            kT = persist.tile([128, 4, S], BF16, tag="kT", name="kT")
            # per token tile: 4 groups of [even(64) | ones(64) | odd(64)]
            vA = persist.tile([128, NT, 768], BF16, tag="vA", name="vA")
            xT = persist.tile([128, 4, S], BF16, tag="xT", name="xT")
            xv_sb = persist.tile([128, 8, S], BF16, tag="xv", name="xv_sb")
            wv_sb = persist.tile([128, 8, G], BF16, tag="wv", name="wv_sb")
            wo_sb = persist.tile([128, 4, D], BF16, tag="wo", name="wo_sb")
            mb = persist.tile([128, 128], F32, tag="mb", name="mb")

            nc.gpsimd.memset(
                vA.rearrange("p t (q c) -> p (t q) c", c=192)[:, :, 64:128], 1.0
            )

            # ---------------- phase 1: Q/K projections ----------------
            with (
                tc.tile_pool(name="p1x", bufs=3) as p1x,
                tc.tile_pool(name="p1w", bufs=2) as p1w,
                tc.tile_pool(name="ps1", bufs=4, space="PSUM") as ps1,
            ):
                with nc.named_scope("proj"):
                    for kind, xd, wd, dest in (("q", xq, wq, qT), ("k", xk, wk, kT)):
                        w_sb = p1w.tile([128, 8, G], BF16, tag="w", name=f"w_{kind}")
                        nc.sync.dma_start(
                            out=w_sb, in_=wd.ap().rearrange("(a p) n -> p a n", p=128)
                        )
                        for tci in range(4):
                            xt = p1x.tile([128, 8, 512], BF16, tag="xt",
                                          name=f"xt_{kind}{tci}")
                            nc.sync.dma_start(
                                out=xt,
                                in_=xd.ap()[:, 512 * tci:512 * tci + 512]
                                .rearrange("(a p) t -> p a t", p=128),
                            )
                            for dq in range(4):
                                acc = ps1.tile([128, 512], F32, tag="pj",
                                               name=f"pj_{kind}{tci}{dq}")
                                for dm in range(8):
                                    nc.tensor.matmul(
                                        acc,
                                        w_sb[:, dm, 128 * dq:128 * dq + 128],
                                        xt[:, dm, :],
                                        start=(dm == 0), stop=(dm == 7),
                                    )
                                nc.scalar.copy(
                                    dest[:, dq, 512 * tci:512 * tci + 512], acc
                                )
                    # V inputs for the in-attention V-projection filler
                    nc.sync.dma_start(
                        out=wv_sb, in_=wv.ap().rearrange("(a p) n -> p a n", p=128)
                    )
                    nc.sync.dma_start(
                        out=xv_sb, in_=xv.ap().rearrange("(a p) t -> p a t", p=128)
                    )
                    nc.sync.dma_start(
                        out=wo_sb, in_=wo.ap().rearrange("(a p) n -> p a n", p=128)
                    )
                    nc.sync.dma_start(out=mb, in_=mb_d.ap())

            # ---------------- phase 2: attention (+ fillers) ----------------
            with (
                tc.tile_pool(name="p2", bufs=1) as p2,
                tc.tile_pool(name="ps2", bufs=1, space="PSUM") as ps2,
            ):
                with nc.named_scope("attn"):
                    prev_mm = [None]

                    def chain(bi):
                        if prev_mm[0] is not None:
                            tile.add_dep_helper(
                                bi.ins, prev_mm[0].ins, sync=False,
                                reason="attn PE order",
                            )
                        prev_mm[0] = bi

                    def vproj(t):
                        # V-proj of token tile t -> vA[:, t, dims blocks]
                        acc = ps2.tile([128, 512], F32, tag="po", bufs=2,
                                       name=f"pv{t}")
                        for dm in range(8):
                            chain(nc.tensor.matmul(
                                acc,
                                xv_sb[:, dm, 128 * t:128 * t + 128],
                                wv_sb[:, dm, :],
                                start=(dm == 0), stop=(dm == 7),
                            ))
                        accv = acc.rearrange("p (q two c) -> p q two c", two=2, c=64)
                        vAt = vA[:, t, :].rearrange("p (q g c) -> p q g c", g=3, c=64)
                        nc.scalar.copy(vAt[:, :, 0, :], accv[:, :, 0, :])
                        nc.scalar.copy(vAt[:, :, 2, :], accv[:, :, 1, :])

                    # O-projection group queues: ("0", ii, n) uses d=0..2 into
                    # out0; ("1", ii, n) uses d=3 into out1.
                    po0_q = [("0", ii, n) for ii in range(NT) for n in range(2)]
                    po1_q = []  # filled as h7 epilogues complete

                    def po_group(which, ii, n):
                        acc = ps2.tile([128, 512], F32, tag="po", bufs=2,
                                       name=f"po{which}_{ii}_{n}")
                        ds = (0, 1, 2) if which == "0" else (3,)
                        for z, d_ in enumerate(ds):
                            chain(nc.tensor.matmul(
                                acc,
                                xT[:, d_, 128 * ii:128 * ii + 128],
                                wo_sb[:, d_, 512 * n:512 * n + 512],
                                start=(z == 0), stop=(z == len(ds) - 1),
                            ))
                        ob = p2.tile([128, 512], F32, tag="ob", bufs=3,
                                     name=f"ob{which}_{ii}_{n}")
                        nc.vector.tensor_scalar_add(ob, acc, 0.0)
                        dst = out0_d if which == "0" else out1_d
                        nc.sync.dma_start(
                            out=dst.ap()[128 * ii:128 * ii + 128,
                                         512 * n:512 * n + 512],
                            in_=ob,
                        )

                    def po_fill(budget):
                        while budget > 0 and (po0_q or po1_q):
                            which, ii, n = (po0_q or po1_q).pop(0)
                            po_group(which, ii, n)
                            budget -= 1

                    for h in range(8):
                        d, par = h // 2, h % 2
                        off = 64 * par
                        kTh = kT[off:off + 64, d, :]
                        qTh = qT[off:off + 64, d, :]
                        for j in range(4):
                            oTj = ps2.tile([128, 512], F32, tag=f"oT{j % 2}",
                                           name=f"oT_{h}_{j}")
                            U = 2 * j + 2
                            pts = {}

                            def av(u):
                                for half in range(2):
                                    kv = 2 * u + half
                                    q0 = max(0, 128 * kv - 512 * j)
                                    lhsT = vA[:, kv,
                                              192 * d + off:192 * d + off + 128]
                                    chain(nc.tensor.matmul(
                                        oTj[:, q0:512],
                                        lhsT,
                                        pts[u][:, 512 * half + q0:512 * half + 512],
                                        start=(kv == 0), stop=(kv == 4 * j + 3),
                                    ))

                            for u in range(U):
                                sb = ps2.tile([128, 1024], F32, tag=f"S{u % 2}",
                                              name=f"s_{h}_{j}_{u}")
                                for half in range(2):
                                    kv = 2 * u + half
                                    q0 = max(0, 128 * kv - 512 * j)
                                    chain(nc.tensor.matmul(
                                        sb[:, 512 * half + q0:512 * half + 512],
                                        kTh[:, 128 * kv:128 * kv + 128],
                                        qTh[:, 512 * j + q0:512 * j + 512],
                                        start=True, stop=True,
                                    ))
                                if u >= U - 2:  # diagonal unit: mask adds
                                    for half in range(2):
                                        kv = 2 * u + half
                                        q0 = 128 * kv - 512 * j
                                        nc.vector.tensor_add(
                                            sb[:, 512 * half + q0:
                                               512 * half + q0 + 128],
                                            sb[:, 512 * half + q0:
                                               512 * half + q0 + 128],
                                            mb,
                                        )
                                if h == 0:
                                    # V-proj filler: j=0 -> 2 tiles/unit,
                                    # j>=1 -> tile 4j+u for u<4
                                    if j == 0:
                                        vproj(2 * u)
                                        vproj(2 * u + 1)
                                    elif u < 4:
                                        vproj(4 * j + u)
                                pt = p2.tile([128, 1024], BF16, tag="pt", bufs=6,
                                             name=f"pt_{h}_{j}_{u}")
                                nc.scalar.activation(pt, sb, EXP, scale=0.125)
                                pts[u] = pt
                                if u >= 2:
                                    av(u - 2)
                                if h >= 6:
                                    po_fill(2)
                            av(U - 2)
                            av(U - 1)

                            # epilogue: normalize chunk j of head h into xT.
                            # recip_approx_fast requires SBUF src at partition
                            # base 0, so stage the denominator rows first.
                            den = p2.tile([64, 512], F32, tag="den", bufs=2,
                                          name=f"den_{h}_{j}")
                            rbc = p2.tile([64, 512], F32, tag="rbc", bufs=2,
                                          name=f"rbc_{h}_{j}")
                            dlo, dhi = ((64, 128) if par == 0 else (0, 64))
                            nc.vector.tensor_scalar_add(
                                den, oTj[dlo:dhi, :], 0.0)
                            nc.vector.reciprocal_approx_fast(rbc, den)
                            xlo, xhi = ((0, 64) if par == 0 else (64, 128))
                            nc.vector.tensor_mul(
                                xT[xlo:xhi, d, 512 * j:512 * j + 512],
                                oTj[xlo:xhi, :], rbc,
                            )
                            if h == 7:
                                # out1 groups for this chunk now computable
                                po1_q.extend(
                                    ("1", ii, n)
                                    for ii in range(4 * j, 4 * j + 4)
                                    for n in range(2)
                                )

                    # ---------------- phase 3: O-projection tail ----------------
                    with nc.named_scope("oproj"):
                        while po0_q or po1_q:
                            which, ii, n = (po0_q or po1_q).pop(0)
                            po_group(which, ii, n)

    nc.compile()
    return nc


_NC = None


def _get_nc():
    global _NC
    if _NC is None:
        _NC = build()
    return _NC


def _make_in_maps(q, k, v, w_q, w_k, w_v, w_o):
    bf = mybir.dt.np(BF16)
    col = np.arange(128)[None, :]
    row = np.arange(128)[:, None]
    mbig = np.where(col >= row, 0.0, NEG).astype(np.float32)

    xqT = [np.ascontiguousarray(np.asarray(q[b]).T).astype(bf) for b in range(B)]
    xkT = [np.ascontiguousarray(np.asarray(k[b]).T).astype(bf) for b in range(B)]
    xvT = [np.ascontiguousarray(np.asarray(v[b]).T).astype(bf) for b in range(B)]
    wqT = [np.ascontiguousarray(np.asarray(w_q[G * g:G * g + G, :]).T).astype(bf)
           for g in range(2)]
    wkT = [np.ascontiguousarray(np.asarray(w_k[G * g:G * g + G, :]).T).astype(bf)
           for g in range(2)]
    wvT = [np.ascontiguousarray(np.asarray(w_v[G * g:G * g + G, :]).T).astype(bf)
           for g in range(2)]
    woT = [np.ascontiguousarray(np.asarray(w_o[:, G * g:G * g + G]).T).astype(bf)
           for g in range(2)]

    in_maps = []
    for c in range(8):
        b, g = c // 2, c % 2
        in_maps.append({
            "xq": xqT[b], "xk": xkT[b], "xv": xvT[b],
            "wq": wqT[g], "wk": wkT[g], "wv": wvT[g], "wo": woT[g],
            "mb": mbig,
        })
    return in_maps


def _gather(results):
    out = np.empty((B, S, D), np.float32)
    for b in range(B):
        out[b] = (results[2 * b]["out0"] + results[2 * b]["out1"]
                  + results[2 * b + 1]["out0"] + results[2 * b + 1]["out1"])
    return out


def run_kernel(inputs, trace=False, tmpdir=None):
    """Run on 8 cores; returns (out, BassKernelResults)."""
    in_maps = _make_in_maps(
        inputs["q"], inputs["k"], inputs["v"],
        inputs["w_q"], inputs["w_k"], inputs["w_v"], inputs["w_o"],
    )
    res = run_bass_kernel_spmd(
        _get_nc(), in_maps, core_ids=list(range(8)), trace=trace, tmpdir=tmpdir
    )
    return _gather(res.results), res


def kernel(**inputs) -> np.ndarray:
    out, _ = run_kernel(inputs)
    return out


# revision 18
# speedup vs baseline: 1.0496x; 1.0169x over previous
"""Multi-head causal attention (B=4, S=2048, D=1024, H=16) on 8 TRN2 cores.

Sharding: core c handles batch c//2 and head-group c%2 (8 heads = 512 dims).
Each core computes its group's QKV projections, causal attention, and two
partial O-projections (out0 = d-blocks 0..2, out1 = d-block 3); the host
sums the four partials per batch.

v2 redesign (baseline was 653us):
- bf16 inputs + weights (halves input DMA); q/k activations kept fp32r.
- attention loops q-chunk OUTER, kv-pair inner. PSUM: 2 score bufs
  [128,1024] + 2 oT accumulators [128,512] + 2 po bufs = 8 banks.
- V tiles carry a shared 64-wide ones block per head pair
  ([even | ones | odd] x 4); the AV matmul then yields 64 numerator rows
  and 64 replicated denominator rows in one pass, so the epilogue is just
  reciprocal_approx_fast + one tensor multiply. No gpsimd broadcast, no
  slow DVE reciprocal, no partition-shift DMA.
- exp always full [128,1024] (stale/garbage columns are never read by AV).
- V-projection runs as PE filler inside head 0; O-projection is split
  out0/out1 and interleaved into heads 6-7 so the PE stays dense enough
  to hold its 2.4GHz p-state (it idles down to 1.2GHz otherwise).
"""

import numpy as np

import concourse.bass as bass
import concourse.mybir as mybir
import concourse.tile as tile
from concourse import bacc
from concourse.bass_utils import run_bass_kernel_spmd

F32 = mybir.dt.float32
F32R = mybir.dt.float32r
BF16 = mybir.dt.bfloat16
EXP = mybir.ActivationFunctionType.Exp

B, S, D = 4, 2048, 1024
G = 512          # dims per head group
NT = S // 128    # 16 token tiles
NEG = -1.0e30


def build():
    nc = bacc.Bacc("TRN2", num_devices=8)

    xq = nc.dram_tensor("xq", [D, S], BF16, kind="ExternalInput")
    xk = nc.dram_tensor("xk", [D, S], BF16, kind="ExternalInput")
    xv = nc.dram_tensor("xv", [D, S], BF16, kind="ExternalInput")
    wq = nc.dram_tensor("wq", [D, G], BF16, kind="ExternalInput")
    wk = nc.dram_tensor("wk", [D, G], BF16, kind="ExternalInput")
    wv = nc.dram_tensor("wv", [D, G], BF16, kind="ExternalInput")
    wo = nc.dram_tensor("wo", [G, D], BF16, kind="ExternalInput")
    mb_d = nc.dram_tensor("mb", [128, 128], F32, kind="ExternalInput")
    # PE-matmul mask factors: dneg.T @ utri adds -1e30 exactly where kl > ql
    dneg_d = nc.dram_tensor("dneg", [128, 128], BF16, kind="ExternalInput")
    utri_d = nc.dram_tensor("utri", [128, 128], BF16, kind="ExternalInput")
    out0_d = nc.dram_tensor("out0", [S, D], F32, kind="ExternalOutput")
    out1_d = nc.dram_tensor("out1", [S, D], F32, kind="ExternalOutput")

    with tile.TileContext(nc) as tc:
        with tc.tile_pool(name="persist", bufs=1) as persist:
            qT = persist.tile([128, 4, S], BF16, tag="qT", name="qT")
            kT = persist.tile([128, 4, S], BF16, tag="kT", name="kT")
            # per token tile: 4 groups of [even(64) | ones(64) | odd(64)]
            vA = persist.tile([128, NT, 768], BF16, tag="vA", name="vA")
            xT = persist.tile([128, 4, S], BF16, tag="xT", name="xT")
            xv_sb = persist.tile([128, 8, S], BF16, tag="xv", name="xv_sb")
            wv_sb = persist.tile([128, 8, G], BF16, tag="wv", name="wv_sb")
            wo_sb = persist.tile([128, 4, D], BF16, tag="wo", name="wo_sb")
            mb = persist.tile([128, 128], F32, tag="mb", name="mb")
            dneg = persist.tile([128, 128], BF16, tag="dneg", name="dneg")
            utri = persist.tile([128, 128], BF16, tag="utri", name="utri")

            nc.gpsimd.memset(
                vA.rearrange("p t (q c) -> p (t q) c", c=192)[:, :, 64:128], 1.0
            )

            # ---------------- phase 1: Q/K projections ----------------
            with (
                tc.tile_pool(name="p1x", bufs=3) as p1x,
                tc.tile_pool(name="p1w", bufs=2) as p1w,
                tc.tile_pool(name="ps1", bufs=4, space="PSUM") as ps1,
            ):
                with nc.named_scope("proj"):
                    for kind, xd, wd, dest in (("q", xq, wq, qT), ("k", xk, wk, kT)):
                        w_sb = p1w.tile([128, 8, G], BF16, tag="w", name=f"w_{kind}")
                        nc.sync.dma_start(
                            out=w_sb, in_=wd.ap().rearrange("(a p) n -> p a n", p=128)
                        )
                        for tci in range(4):
                            xt = p1x.tile([128, 8, 512], BF16, tag="xt",
                                          name=f"xt_{kind}{tci}")
                            nc.sync.dma_start(
                                out=xt,
                                in_=xd.ap()[:, 512 * tci:512 * tci + 512]
                                .rearrange("(a p) t -> p a t", p=128),
                            )
                            for dq in range(4):
                                acc = ps1.tile([128, 512], F32, tag="pj",
                                               name=f"pj_{kind}{tci}{dq}")
                                for dm in range(8):
                                    nc.tensor.matmul(
                                        acc,
                                        w_sb[:, dm, 128 * dq:128 * dq + 128],
                                        xt[:, dm, :],
                                        start=(dm == 0), stop=(dm == 7),
                                    )
                                nc.scalar.copy(
                                    dest[:, dq, 512 * tci:512 * tci + 512], acc
                                )
                    # V inputs for the in-attention V-projection filler
                    nc.sync.dma_start(
                        out=wv_sb, in_=wv.ap().rearrange("(a p) n -> p a n", p=128)
                    )
                    nc.sync.dma_start(
                        out=xv_sb, in_=xv.ap().rearrange("(a p) t -> p a t", p=128)
                    )
                    nc.sync.dma_start(
                        out=wo_sb, in_=wo.ap().rearrange("(a p) n -> p a n", p=128)
                    )
                    nc.sync.dma_start(out=mb, in_=mb_d.ap())
                    nc.sync.dma_start(out=dneg, in_=dneg_d.ap())
                    nc.sync.dma_start(out=utri, in_=utri_d.ap())

            # ---------------- phase 2: attention (+ fillers) ----------------
            with (
                tc.tile_pool(name="p2", bufs=1) as p2,
                tc.tile_pool(name="ps2", bufs=1, space="PSUM") as ps2,
            ):
                with nc.named_scope("attn"):
                    prev_mm = [None]

                    def chain(bi):
                        if prev_mm[0] is not None:
                            tile.add_dep_helper(
                                bi.ins, prev_mm[0].ins, sync=False,
                                reason="attn PE order",
                            )
                        prev_mm[0] = bi

                    def vproj(t):
                        # V-proj of token tile t -> vA[:, t, dims blocks]
                        acc = ps2.tile([128, 512], F32, tag="po", bufs=2,
                                       name=f"pv{t}")
                        for dm in range(8):
                            chain(nc.tensor.matmul(
                                acc,
                                xv_sb[:, dm, 128 * t:128 * t + 128],
                                wv_sb[:, dm, :],
                                start=(dm == 0), stop=(dm == 7),
                            ))
                        accv = acc.rearrange("p (q two c) -> p q two c", two=2, c=64)
                        vAt = vA[:, t, :].rearrange("p (q g c) -> p q g c", g=3, c=64)
                        nc.scalar.copy(vAt[:, :, 0, :], accv[:, :, 0, :])
                        nc.scalar.copy(vAt[:, :, 2, :], accv[:, :, 1, :])

                    # O-projection group queues: ("0", ii, n) uses d=0..2 into
                    # out0; ("1", ii, n) uses d=3 into out1.
                    po0_q = [("0", ii, n) for ii in range(NT) for n in range(2)]
                    po1_q = []  # filled as h7 epilogues complete

                    def po_group(which, ii, n):
                        acc = ps2.tile([128, 512], F32, tag="po", bufs=2,
                                       name=f"po{which}_{ii}_{n}")
                        ds = (0, 1, 2) if which == "0" else (3,)
                        for z, d_ in enumerate(ds):
                            chain(nc.tensor.matmul(
                                acc,
                                xT[:, d_, 128 * ii:128 * ii + 128],
                                wo_sb[:, d_, 512 * n:512 * n + 512],
                                start=(z == 0), stop=(z == len(ds) - 1),
                            ))
                        ob = p2.tile([128, 512], F32, tag="ob", bufs=3,
                                     name=f"ob{which}_{ii}_{n}")
                        nc.vector.tensor_scalar_add(ob, acc, 0.0)
                        dst = out0_d if which == "0" else out1_d
                        nc.sync.dma_start(
                            out=dst.ap()[128 * ii:128 * ii + 128,
                                         512 * n:512 * n + 512],
                            in_=ob,
                        )

                    def po_fill(budget):
                        while budget > 0 and (po0_q or po1_q):
                            which, ii, n = (po0_q or po1_q).pop(0)
                            po_group(which, ii, n)
                            budget -= 1

                    for h in range(8):
                        d, par = h // 2, h % 2
                        off = 64 * par
                        kTh = kT[off:off + 64, d, :]
                        qTh = qT[off:off + 64, d, :]
                        for j in range(4):
                            oTj = ps2.tile([128, 512], F32, tag=f"oT{j % 2}",
                                           name=f"oT_{h}_{j}")
                            U = 2 * j + 2
                            pts = {}

                            def av(u):
                                for half in range(2):
                                    kv = 2 * u + half
                                    q0 = max(0, 128 * kv - 512 * j)
                                    lhsT = vA[:, kv,
                                              192 * d + off:192 * d + off + 128]
                                    chain(nc.tensor.matmul(
                                        oTj[:, q0:512],
                                        lhsT,
                                        pts[u][:, 512 * half + q0:512 * half + 512],
                                        start=(kv == 0), stop=(kv == 4 * j + 3),
                                    ))

                            for u in range(U):
                                sb = ps2.tile([128, 1024], F32, tag=f"S{u % 2}",
                                              name=f"s_{h}_{j}_{u}")
                                diag_unit = u >= U - 2
                                # early chunks: mask on PE so head-start exps
                                # aren't gated behind the DVE epilogue queue
                                pe_mask = diag_unit and j <= 1
                                for half in range(2):
                                    kv = 2 * u + half
                                    q0 = max(0, 128 * kv - 512 * j)
                                    chain(nc.tensor.matmul(
                                        sb[:, 512 * half + q0:512 * half + 512],
                                        kTh[:, 128 * kv:128 * kv + 128],
                                        qTh[:, 512 * j + q0:512 * j + 512],
                                        start=True, stop=not pe_mask,
                                    ))
                                    if pe_mask:
                                        chain(nc.tensor.matmul(
                                            sb[:, 512 * half + q0:
                                               512 * half + q0 + 128],
                                            dneg, utri,
                                            start=False, stop=True,
                                        ))
                                if diag_unit and not pe_mask:
                                    for half in range(2):
                                        kv = 2 * u + half
                                        q0 = 128 * kv - 512 * j
                                        nc.vector.tensor_add(
                                            sb[:, 512 * half + q0:
                                               512 * half + q0 + 128],
                                            sb[:, 512 * half + q0:
                                               512 * half + q0 + 128],
                                            mb,
                                        )
                                if h == 0:
                                    # V-proj filler: j=0 -> 2 tiles/unit,
                                    # j>=1 -> tile 4j+u for u<4
                                    if j == 0:
                                        vproj(2 * u)
                                        vproj(2 * u + 1)
                                    elif u < 4:
                                        vproj(4 * j + u)
                                pt = p2.tile([128, 1024], BF16, tag="pt", bufs=6,
                                             name=f"pt_{h}_{j}_{u}")
                                nc.scalar.activation(pt, sb, EXP, scale=0.125)
                                pts[u] = pt
                                if u >= 2:
                                    av(u - 2)
                                if h >= 6:
                                    po_fill(2)
                            av(U - 2)
                            av(U - 1)

                            # epilogue: normalize chunk j of head h into xT.
                            # recip_approx_fast requires SBUF src at partition
                            # base 0, so stage the denominator rows first.
                            den = p2.tile([64, 512], F32, tag="den", bufs=2,
                                          name=f"den_{h}_{j}")
                            rbc = p2.tile([64, 512], F32, tag="rbc", bufs=2,
                                          name=f"rbc_{h}_{j}")
                            dlo, dhi = ((64, 128) if par == 0 else (0, 64))
                            nc.vector.tensor_scalar_add(
                                den, oTj[dlo:dhi, :], 0.0)
                            nc.vector.reciprocal_approx_fast(rbc, den)
                            xlo, xhi = ((0, 64) if par == 0 else (64, 128))
                            nc.vector.tensor_mul(
                                xT[xlo:xhi, d, 512 * j:512 * j + 512],
                                oTj[xlo:xhi, :], rbc,
                            )
                            if h == 7:
                                # out1 groups for this chunk now computable
                                po1_q.extend(
                                    ("1", ii, n)
                                    for ii in range(4 * j, 4 * j + 4)
                                    for n in range(2)
                                )

                    # ---------------- phase 3: O-projection tail ----------------
                    with nc.named_scope("oproj"):
                        while po0_q or po1_q:
                            which, ii, n = (po0_q or po1_q).pop(0)
                            po_group(which, ii, n)

    nc.compile()
    return nc


_NC = None


def _get_nc():
    global _NC
    if _NC is None:
        _NC = build()
    return _NC


def _make_in_maps(q, k, v, w_q, w_k, w_v, w_o):
    bf = mybir.dt.np(BF16)
    col = np.arange(128)[None, :]
    row = np.arange(128)[:, None]
    mbig = np.where(col >= row, 0.0, NEG).astype(np.float32)
    dneg = (np.eye(128, dtype=np.float32) * NEG).astype(bf)
    utri = (np.arange(128)[:, None] > np.arange(128)[None, :]).astype(bf)

    xqT = [np.ascontiguousarray(np.asarray(q[b]).T).astype(bf) for b in range(B)]
    xkT = [np.ascontiguousarray(np.asarray(k[b]).T).astype(bf) for b in range(B)]
    xvT = [np.ascontiguousarray(np.asarray(v[b]).T).astype(bf) for b in range(B)]
    wqT = [np.ascontiguousarray(np.asarray(w_q[G * g:G * g + G, :]).T).astype(bf)
           for g in range(2)]
    wkT = [np.ascontiguousarray(np.asarray(w_k[G * g:G * g + G, :]).T).astype(bf)
           for g in range(2)]
    wvT = [np.ascontiguousarray(np.asarray(w_v[G * g:G * g + G, :]).T).astype(bf)
           for g in range(2)]
    woT = [np.ascontiguousarray(np.asarray(w_o[:, G * g:G * g + G]).T).astype(bf)
           for g in range(2)]

    in_maps = []
    for c in range(8):
        b, g = c // 2, c % 2
        in_maps.append({
            "xq": xqT[b], "xk": xkT[b], "xv": xvT[b],
            "wq": wqT[g], "wk": wkT[g], "wv": wvT[g], "wo": woT[g],
            "mb": mbig, "dneg": dneg, "utri": utri,
        })
    return in_maps


def _gather(results):
    out = np.empty((B, S, D), np.float32)
    for b in range(B):
        out[b] = (results[2 * b]["out0"] + results[2 * b]["out1"]
                  + results[2 * b + 1]["out0"] + results[2 * b + 1]["out1"])
    return out


def run_kernel(inputs, trace=False, tmpdir=None):
    """Run on 8 cores; returns (out, BassKernelResults)."""
    in_maps = _make_in_maps(
        inputs["q"], inputs["k"], inputs["v"],
        inputs["w_q"], inputs["w_k"], inputs["w_v"], inputs["w_o"],
    )
    res = run_bass_kernel_spmd(
        _get_nc(), in_maps, core_ids=list(range(8)), trace=trace, tmpdir=tmpdir
    )
    return _gather(res.results), res


def kernel(**inputs) -> np.ndarray:
    out, _ = run_kernel(inputs)
    return out
